# revision 7
# baseline (speedup 1.0000x reference)
"""Trainium2 Bass kernel for GQA attention (B=1, S=2048, D=2048, H=32, KV=8, HD=64).

Tensor-parallel over heads across 8 NeuronCores: core i holds q-heads
[4i, 4i+4) and kv-head i; each core computes its partial o_proj output and the
host sums the 8 partials (Megatron all-reduce done host-side).

Structure (v2):
 - P1: qkv projections per 512-col s-block + RoPE (bf16, spread over
   ACT/DVE/GpSimd) + interleaved attention for the head-pair (0,1).
 - P2: attention for head-pair (2,3) + (0,1) qb=3, with o_proj s-blocks
   interleaved as soon as their q-block is fully normalized, so the PE fills
   the idle left by the ACT-bound softmax exp.
 - Score matmuls pack 2 heads per PE pass (64-contraction row tiles).
 - Softmax normalization fused into the PSUM eviction (reciprocal + gpsimd
   partition_broadcast + one tensor_tensor).

Self-contained: only imports concourse (on sys.path in the container).
"""

import os
import sys

import ml_dtypes
import numpy as np

if "/opt/trn_rl_repo" not in sys.path and not any(
    p.endswith("trn_rl_repo") for p in sys.path
):
    sys.path.insert(0, "/opt/trn_rl_repo")

import concourse.bass as bass
import concourse.mybir as mybir
import concourse.tile as tile
from concourse import bacc
from concourse.bass_utils import run_bass_kernel_spmd
from concourse.masks import make_identity

F32 = mybir.dt.float32
F32R = mybir.dt.float32r
BF16 = mybir.dt.bfloat16

AF = mybir.ActivationFunctionType
ALU = mybir.AluOpType

S = 2048
D = 2048
H = 32
KV = 8
HD = 64
NCORES = 8
HQ = H // NCORES  # 4 q heads per core (2 head-pairs)
NKC = S // 128  # 16 key chunks
NQB = 4  # q blocks of 512
QBW = 512
NSB = 4  # s blocks of 512 in projection
SBW = 512
DCH = D // 128  # 16 contraction chunks


def _build_nc():
    nc = bacc.Bacc("TRN2", target_bir_lowering=False, debug=False, num_devices=NCORES)

    xt_d = nc.declare_dram_parameter("xt", [D, S], BF16, isOutput=False)
    wqkv_d = nc.declare_dram_parameter("wqkv", [D, 384], BF16, isOutput=False)
    wo_d = nc.declare_dram_parameter("wo", [2, 128, D], BF16, isOutput=False)
    cos_d = nc.declare_dram_parameter("cos", [128, S], BF16, isOutput=False)
    sin_d = nc.declare_dram_parameter("sin", [128, S], BF16, isOutput=False)
    y_d = nc.declare_dram_parameter("y", [S, D], BF16, isOutput=True)
    kdbg = bool(os.environ.get("KDBG"))
    if kdbg:
        dbg_qs_d = nc.declare_dram_parameter("dbg_qs", [128, 2, S], BF16, isOutput=True)
        dbg_kt_d = nc.declare_dram_parameter("dbg_kt", [128, S], BF16, isOutput=True)
        dbg_vs_d = nc.declare_dram_parameter(
            "dbg_vs", [128, NKC * 65], BF16, isOutput=True
        )
        dbg_ao_d = nc.declare_dram_parameter("dbg_ao", [128, 2, S], BF16, isOutput=True)

    with tile.TileContext(nc) as tc:
        with tc.tile_pool(name="glob", bufs=1) as glob:
            ktdup = glob.tile([128, S], BF16, tag="ktdup")
            v_s = glob.tile([128, NKC, 65], BF16, tag="v_s")
            outA = glob.tile([128, S], BF16, tag="outA")
            outB = glob.tile([128, S], BF16, tag="outB")
            ao = glob.tile([128, 2, S], BF16, tag="ao")
            ident = glob.tile([128, 128], BF16, tag="ident")
            wo_s = glob.tile([128, 2, D], BF16, tag="wo_s")
            cos_s = glob.tile([128, S], BF16, tag="cos_s")
            sin_s = glob.tile([128, S], BF16, tag="sin_s")
            wq_s = glob.tile([128, DCH, 384], BF16, tag="wq_s")
            kvraw = glob.tile([128, S], BF16, tag="kvraw")
            kswap = glob.tile([64, S], BF16, tag="kswap")
            qs01 = glob.tile([128, S], BF16, tag="qs01")
            qs23 = glob.tile([128, S], BF16, tag="qs23")
            rcp_t = glob.tile([128, 2, QBW], F32, tag="rcp_t")
            rcp_b = glob.tile([128, 2, QBW], BF16, tag="rcp_b")
            ones_b = glob.tile([128, 128], BF16, tag="ones_b")

            nc.vector.memset(v_s[:, :, 64], 1.0)
            nc.vector.memset(ones_b[:], 1.0)

            with (
                tc.tile_pool(name="xp", bufs=3) as xp,
                tc.tile_pool(name="aab", bufs=4) as aab,
                tc.tile_pool(name="tmpp", bufs=4) as tmpp,
                tc.tile_pool(name="pttp", bufs=6) as pttp,
                tc.tile_pool(name="stgp", bufs=4) as stgp,
                tc.tile_pool(name="yp", bufs=4) as yp,
                tc.tile_pool(name="psc", bufs=2, space="PSUM") as pscp,
                tc.tile_pool(name="pso", bufs=2, space="PSUM") as psop,
            ):
                dma_engs = [nc.sync, nc.gpsimd]

                def emit_unit(pair, qb, qs, hooks=()):
                    """Attention for head-pair `pair` (0→heads 0,1; 1→heads 2,3),
                    q-block qb.  hooks: callables interleaved between chunks."""
                    q0 = qb * QBW
                    nkc = 4 * (qb + 1)
                    hooks = list(hooks)
                    pso_E = psop.tile([128, QBW], F32, tag="pso")
                    pso_O = psop.tile([128, QBW], F32, tag="pso")
                    hook_every = max(1, nkc // max(1, len(hooks))) if hooks else 0
                    hi = 0
                    for c in range(nkc):
                        kc0 = c * 128
                        d = max(0, kc0 - q0)
                        psc = pscp.tile([128, 1024], F32, tag="psc")
                        ptt = pttp.tile([128, 1024], BF16, tag="ptt")
                        # two heads concurrently (row tiles 0-63 / 64-127)
                        nc.tensor.matmul(
                            psc[:, d:512],
                            lhsT=ktdup[0:64, kc0 : kc0 + 128],
                            rhs=qs[0:64, q0 + d : q0 + QBW],
                            start=True,
                            stop=True,
                            tile_position=(0, 0),
                        )
                        nc.tensor.matmul(
                            psc[:, 512 + d : 1024],
                            lhsT=ktdup[64:128, kc0 : kc0 + 128],
                            rhs=qs[64:128, q0 + d : q0 + QBW],
                            start=True,
                            stop=True,
                            tile_position=(64, 0),
                        )
                        if d == 0:
                            nc.scalar.activation(ptt[:, 0:1024], psc[:, 0:1024], AF.Exp)
                        else:
                            nc.scalar.activation(ptt[:, d:512], psc[:, d:512], AF.Exp)
                            nc.scalar.activation(
                                ptt[:, 512 + d : 1024], psc[:, 512 + d : 1024], AF.Exp
                            )
                        if kc0 + 127 > q0:
                            ww = min(512, (kc0 - q0) + 128)
                            for half in (0, 1):
                                sl = slice(half * 512 + d, half * 512 + ww)
                                nc.gpsimd.affine_select(
                                    out=ptt[:, sl],
                                    in_=ptt[:, sl],
                                    compare_op=ALU.is_ge,
                                    fill=0.0,
                                    base=q0 - kc0 + d,
                                    channel_multiplier=-1,
                                    pattern=[[1, ww - d]],
                                )
                        nc.tensor.matmul(
                            pso_E[0:65, d:QBW],
                            lhsT=v_s[:, c, :],
                            rhs=ptt[:, d:512],
                            start=(c == 0),
                            stop=(c == nkc - 1),
                        )
                        nc.tensor.matmul(
                            pso_O[0:65, d:QBW],
                            lhsT=v_s[:, c, :],
                            rhs=ptt[:, 512 + d : 1024],
                            start=(c == 0),
                            stop=(c == nkc - 1),
                        )
                        if hooks and hi < len(hooks) and (c % hook_every == hook_every - 1):
                            hooks[hi]()
                            hi += 1
                    while hi < len(hooks):
                        hooks[hi]()
                        hi += 1
                    # fused normalize + eviction (DVE reciprocal, f32r broadcast)
                    ch = pair
                    qsl = slice(q0, q0 + QBW)
                    bcps = pscp.tile([128, 1024], F32, tag="psc")
                    nc.vector.reciprocal(rcp_t[64:65, 0, :], pso_E[64:65, :])
                    nc.vector.tensor_copy(rcp_b[64:65, 0, :], rcp_t[64:65, 0, :])
                    nc.tensor.matmul(
                        bcps[:, 0:512],
                        lhsT=ones_b[64:65, :],
                        rhs=rcp_b[64:65, 0, :],
                        start=True,
                        stop=True,
                    )
                    bcE_s = stgp.tile([64, QBW], F32, tag="bcs")
                    nc.vector.tensor_copy(bcE_s[0:64, :], bcps[0:64, 0:512])
                    nc.vector.tensor_tensor(
                        ao[0:64, ch, qsl], pso_E[0:64, :], bcE_s[0:64, :], ALU.mult
                    )
                    nc.vector.reciprocal(rcp_t[64:65, 1, :], pso_O[64:65, :])
                    nc.vector.tensor_copy(rcp_b[64:65, 1, :], rcp_t[64:65, 1, :])
                    nc.tensor.matmul(
                        bcps[:, 512:1024],
                        lhsT=ones_b[64:65, :],
                        rhs=rcp_b[64:65, 1, :],
                        start=True,
                        stop=True,
                    )
                    bcO_s = stgp.tile([64, QBW], F32, tag="bcs")
                    nc.vector.tensor_copy(bcO_s[0:64, :], bcps[0:64, 512:1024])
                    stg = stgp.tile([64, QBW], BF16, tag="stg")
                    nc.vector.tensor_tensor(
                        stg[0:64, :], pso_O[0:64, :], bcO_s[0:64, :], ALU.mult
                    )
                    nc.gpsimd.dma_start(ao[64:128, ch, qsl], stg[0:64, :])

                def oproj_half(st, hb):
                    """o_proj for s-chunk st (128 rows of y), half hb (1024 cols)."""
                    ysb = yp.tile([128, 1024], BF16, tag="ysb")
                    for obi in (0, 1):
                        ob = 2 * hb + obi
                        pt = psyp.tile([128, QBW], F32, tag="psy")
                        for chn in (0, 1):
                            nc.tensor.matmul(
                                pt[:],
                                lhsT=ao[:, chn, st * 128 : (st + 1) * 128],
                                rhs=wo_s[:, chn, ob * 512 : (ob + 1) * 512],
                                start=(chn == 0),
                                stop=(chn == 1),
                            )
                        dst = ysb[:, obi * 512 : (obi + 1) * 512]
                        if (st + ob) % 3 < 2:
                            nc.vector.tensor_copy(dst, pt[:])
                        else:
                            nc.scalar.activation(dst, pt[:], AF.Copy)
                    eng = dma_engs[(st + hb) % 2]
                    eng.dma_start(
                        y_d[st * 128 : (st + 1) * 128, hb * 1024 : (hb + 1) * 1024],
                        ysb[:],
                    )

                # ---------------- P1: projections + RoPE + pair-01 attention ----
                with tc.tile_pool(name="ps1", bufs=2, space="PSUM") as ps1:
                    xt_r = xt_d.rearrange("(ko p) s -> p ko s", p=128)
                    wqkv_r = wqkv_d.rearrange("(ko p) n -> p ko n", p=128)
                    nc.sync.dma_start(cos_s[:, 0:SBW], cos_d[:, 0:SBW])
                    nc.gpsimd.dma_start(sin_s[:, 0:SBW], sin_d[:, 0:SBW])
                    make_identity(nc, ident[:])
                    for sb in range(NSB):
                        sbc = slice(sb * SBW, (sb + 1) * SBW)
                        xblk = xp.tile([128, DCH, SBW], BF16, tag="xblk")
                        for kc in range(DCH):
                            if sb == 0:
                                nc.gpsimd.dma_start(wq_s[:, kc, :], wqkv_r[:, kc, :])
                            nc.sync.dma_start(
                                xblk[:, kc, :], xt_r[:, kc, sbc]
                            )
                        if sb == 0:
                            for chn in range(2):
                                nc.gpsimd.dma_start(wo_s[:, chn, :], wo_d[chn])
                        if sb < NSB - 1:
                            nsbc = slice((sb + 1) * SBW, (sb + 2) * SBW)
                            nc.sync.dma_start(cos_s[:, nsbc], cos_d[:, nsbc])
                            nc.gpsimd.dma_start(sin_s[:, nsbc], sin_d[:, nsbc])
                        psA = ps1.tile([128, SBW], F32, tag="proj")
                        psB = ps1.tile([128, SBW], F32, tag="proj")
                        for ps_t, col0 in ((psA, 0), (psB, 128)):
                            for kc in range(DCH):
                                nc.tensor.matmul(
                                    ps_t[:],
                                    lhsT=wq_s[:, kc, col0 : col0 + 128],
                                    rhs=xblk[:, kc, :],
                                    start=(kc == 0),
                                    stop=(kc == DCH - 1),
                                )
                        aA = aab.tile([128, SBW], BF16, tag="aab")
                        nc.scalar.activation(aA[:], psA[:], AF.Copy)
                        aB = aab.tile([128, SBW], BF16, tag="aab")
                        nc.scalar.activation(aB[:], psB[:], AF.Copy)
                        psKV = ps1.tile([128, SBW], F32, tag="proj")
                        for kc in range(DCH):
                            nc.tensor.matmul(
                                psKV[:],
                                lhsT=wq_s[:, kc, 256:384],
                                rhs=xblk[:, kc, :],
                                start=(kc == 0),
                                stop=(kc == DCH - 1),
                            )
                        # RoPE on q (A = first-half dims, B = second halves)
                        tmp1 = tmpp.tile([128, SBW], BF16, tag="tmp")
                        tmp2 = tmpp.tile([128, SBW], BF16, tag="tmp")
                        nc.vector.tensor_tensor(
                            outA[:, sbc], aA[:], cos_s[:, sbc], ALU.mult
                        )
                        nc.gpsimd.tensor_tensor(
                            tmp1[:], aB[:], sin_s[:, sbc], ALU.mult
                        )
                        nc.vector.tensor_tensor(
                            outA[:, sbc], outA[:, sbc], tmp1[:], ALU.subtract
                        )
                        nc.vector.tensor_tensor(
                            outB[:, sbc], aB[:], cos_s[:, sbc], ALU.mult
                        )
                        nc.gpsimd.tensor_tensor(
                            tmp2[:], aA[:], sin_s[:, sbc], ALU.mult
                        )
                        nc.vector.tensor_tensor(
                            outB[:, sbc], outB[:, sbc], tmp2[:], ALU.add
                        )
                        # k|v eviction; k RoPE via swapped halves
                        nc.scalar.activation(kvraw[:, sbc], psKV[:], AF.Copy)
                        nc.sync.dma_start(kswap[0:32, sbc], kvraw[32:64, sbc])
                        nc.sync.dma_start(kswap[32:64, sbc], kvraw[0:32, sbc])
                        tmpk = tmpp.tile([64, SBW], BF16, tag="tmpk")
                        nc.vector.tensor_tensor(
                            ktdup[0:64, sbc], kvraw[0:64, sbc], cos_s[0:64, sbc],
                            ALU.mult,
                        )
                        nc.gpsimd.tensor_tensor(
                            tmpk[:], kswap[:, sbc], sin_s[0:64, sbc], ALU.mult
                        )
                        nc.vector.tensor_tensor(
                            ktdup[0:32, sbc], ktdup[0:32, sbc], tmpk[0:32, :],
                            ALU.subtract,
                        )
                        nc.vector.tensor_tensor(
                            ktdup[32:64, sbc], ktdup[32:64, sbc], tmpk[32:64, :],
                            ALU.add,
                        )
                        nc.sync.dma_start(ktdup[64:128, sbc], ktdup[0:64, sbc])
                        # v: [64, 512] -> 4 key-chunk tiles [128, 64] via PE transpose
                        for c in range(4 * sb, 4 * sb + 4):
                            ptr = pscp.tile([128, 1024], F32, tag="psc")
                            ptrb = ptr.bitcast(BF16)
                            nc.tensor.transpose(
                                ptrb[:, 0:64],
                                kvraw[64:128, c * 128 : (c + 1) * 128],
                                ident[64:128, 64:128],
                            )
                            nc.vector.tensor_copy(v_s[:, c, 0:64], ptrb[:, 0:64])
                        # head-pair 01 q stream for this s-block
                        nc.scalar.dma_start(qs01[0:32, sbc], outA[0:32, sbc])
                        nc.scalar.dma_start(qs01[32:64, sbc], outB[0:32, sbc])
                        nc.scalar.dma_start(qs01[64:96, sbc], outA[32:64, sbc])
                        nc.scalar.dma_start(qs01[96:128, sbc], outB[32:64, sbc])
                        if sb >= 1:
                            emit_unit(0, sb - 1, qs01)

                # ---------------- P2: pair-01 qb3, pair-23, o_proj interleaved --
                with tc.tile_pool(name="psy", bufs=2, space="PSUM") as psyp:
                    for j in range(4):
                        nc.scalar.dma_start(
                            qs23[32 * j : 32 * j + 32, :],
                            (outA if j % 2 == 0 else outB)[
                                64 + 32 * (j // 2) : 96 + 32 * (j // 2), :
                            ],
                        )

                    def op_hooks(st0, st1):
                        return [
                            lambda s=s, h=h: oproj_half(s, h)
                            for s in range(st0, st1)
                            for h in (0, 1)
                        ]

                    emit_unit(0, 3, qs01)
                    emit_unit(1, 0, qs23)
                    emit_unit(1, 1, qs23, hooks=op_hooks(0, 4))
                    emit_unit(1, 2, qs23, hooks=op_hooks(4, 8))
                    emit_unit(1, 3, qs23, hooks=op_hooks(8, 12))
                    for st in range(12, 16):
                        for hb in (0, 1):
                            oproj_half(st, hb)
                    if kdbg:
                        nc.sync.dma_start(dbg_qs_d[:, 0, :], qs01[:])
                        nc.sync.dma_start(dbg_qs_d[:, 1, :], qs23[:])
                        nc.sync.dma_start(dbg_kt_d[:], ktdup[:])
                        nc.sync.dma_start(
                            dbg_vs_d.rearrange("p (c n) -> p c n", c=NKC), v_s[:]
                        )
                        nc.sync.dma_start(dbg_ao_d[:], ao[:])
    nc.compile()
    return nc


def _prep_inputs(x, Wq, Wk, Wv, Wo, inv_freq):
    """Host-side sharding + layout prep. Returns in_maps for the 8 cores."""
    x = np.ascontiguousarray(np.asarray(x, dtype=np.float32).reshape(S, D))
    xt = np.ascontiguousarray(x.T)  # [D, S]

    pos = np.arange(S, dtype=np.float64)
    inv = np.asarray(inv_freq, dtype=np.float64)  # [32]
    freqs = pos[None, :] * inv[:, None]  # [32, S]
    cos32 = np.cos(freqs).astype(np.float32)
    sin32 = np.sin(freqs).astype(np.float32)
    cos_tab = np.tile(cos32, (4, 1))  # [128, S]
    sin_tab = np.tile(sin32, (4, 1))

    in_maps = []
    for i in range(NCORES):
        wq_l = Wq[256 * i : 256 * (i + 1)].astype(np.float32) * 0.125  # [256, D]
        wk_l = Wk[64 * i : 64 * (i + 1)].astype(np.float32)  # [64, D]
        wv_l = Wv[64 * i : 64 * (i + 1)].astype(np.float32)  # [64, D]
        # A-tile: first-half dims of the 4 heads; B-tile: second halves
        wA = np.concatenate(
            [wq_l[64 * h : 64 * h + 32] for h in range(HQ)], axis=0
        )  # [128, D]
        wB = np.concatenate(
            [wq_l[64 * h + 32 : 64 * h + 64] for h in range(HQ)], axis=0
        )
        wkv = np.concatenate([wk_l, wv_l], axis=0)  # [128, D]
        wqkv = np.ascontiguousarray(
            np.concatenate([wA, wB, wkv], axis=0).T
        )  # [D, 384]
        wo_l = Wo[:, 256 * i : 256 * (i + 1)].astype(np.float32)  # [D, 256]
        wo_t = np.ascontiguousarray(wo_l.T.reshape(2, 128, D))  # [2, 128, D]
        in_maps.append(
            {
                "xt": xt.astype(ml_dtypes.bfloat16),
                "wqkv": wqkv.astype(ml_dtypes.bfloat16),
                "wo": wo_t.astype(ml_dtypes.bfloat16),
                "cos": cos_tab.astype(ml_dtypes.bfloat16),
                "sin": sin_tab.astype(ml_dtypes.bfloat16),
            }
        )
    return in_maps


_NC_CACHE = None


def kernel(x, Wq, Wk, Wv, Wo, inv_freq):
    global _NC_CACHE
    if _NC_CACHE is None:
        _NC_CACHE = _build_nc()
    nc = _NC_CACHE
    in_maps = _prep_inputs(x, Wq, Wk, Wv, Wo, inv_freq)
    trace = bool(int(os.environ.get("BASS_KERNEL_TRACE", "0")))
    res = None
    last_exc = None
    for attempt in range(3):
        try:
            res = run_bass_kernel_spmd(nc, in_maps, list(range(NCORES)), trace=trace)
            break
        except Exception as e:  # transient device faults (rare) — retry
            last_exc = e
            msg = str(e)
            if "UNRECOVERABLE" in msg or "UNAVAILABLE" in msg or "Timeout" in msg:
                continue
            raise
    if res is None:
        raise last_exc
    if trace:
        kernel.last_results = res
    y = np.zeros((S, D), dtype=np.float32)
    for i in range(NCORES):
        y += res.results[i]["y"].astype(np.float32)
    return y.reshape(1, S, D)



# revision 13
# speedup vs baseline: 1.2750x; 1.2750x over previous
"""Trainium2 Bass kernel for GQA attention (B=1, S=2048, D=2048, H=32, KV=8, HD=64).

Tensor-parallel over heads across 8 NeuronCores: core i holds q-heads
[4i, 4i+4) and kv-head i; each core computes its partial o_proj output and the
host sums the 8 partials (Megatron all-reduce done host-side).

Structure (v3b):
 - Single software-pipelined stream: per s-block step, attention units for
   BOTH head-pairs at qb=sb-1 run with the next projection's matmuls and the
   previous q-block's o_proj interleaved as hooks, keeping the PE dense.
 - Deferred softmax-normalize: each unit's normalize tail is emitted inside
   the NEXT unit (after its first chunk), so the PE never waits on the
   reciprocal chain at unit boundaries.
 - Normalize uses DVE reciprocal_approx_fast (full-partition; base-partition-0
   requirement) + bf16 ones-matmul broadcast. No ACT table switches.
 - exp is one 3D-AP ACTIVATE per key-chunk covering both heads.
 - Queue discipline: sync queue carries input DMAs only; stg/y DMAs ride the
   producing engine's queue; x loads batched 4x[128,4,512] and prefetched one
   step ahead.

Self-contained: only imports concourse (on sys.path in the container).
"""

import os
import sys

import ml_dtypes
import numpy as np

if "/opt/trn_rl_repo" not in sys.path and not any(
    p.endswith("trn_rl_repo") for p in sys.path
):
    sys.path.insert(0, "/opt/trn_rl_repo")

import concourse.bass as bass
import concourse.mybir as mybir
import concourse.tile as tile
from concourse import bacc
from concourse.bass_utils import run_bass_kernel_spmd
from concourse.masks import make_identity

F32 = mybir.dt.float32
F32R = mybir.dt.float32r
BF16 = mybir.dt.bfloat16

AF = mybir.ActivationFunctionType
ALU = mybir.AluOpType

S = 2048
D = 2048
H = 32
KV = 8
HD = 64
NCORES = 8
HQ = H // NCORES  # 4 q heads per core (2 head-pairs)
NKC = S // 128  # 16 key chunks
NQB = 4  # q blocks of 512
QBW = 512
NSB = 4  # s blocks of 512 in projection
SBW = 512
DCH = D // 128  # 16 contraction chunks


def _build_nc():
    nc = bacc.Bacc("TRN2", target_bir_lowering=False, debug=False, num_devices=NCORES)

    xt_d = nc.declare_dram_parameter("xt", [D, S], BF16, isOutput=False)
    wqkv_d = nc.declare_dram_parameter("wqkv", [D, 384], BF16, isOutput=False)
    wo_d = nc.declare_dram_parameter("wo", [2, 128, D], BF16, isOutput=False)
    cos_d = nc.declare_dram_parameter("cos", [128, S], BF16, isOutput=False)
    sin_d = nc.declare_dram_parameter("sin", [128, S], BF16, isOutput=False)
    y_d = nc.declare_dram_parameter("y", [S, D], BF16, isOutput=True)
    kdbg = bool(os.environ.get("KDBG"))
    if kdbg:
        dbg_qs_d = nc.declare_dram_parameter("dbg_qs", [128, 2, S], BF16, isOutput=True)
        dbg_kt_d = nc.declare_dram_parameter("dbg_kt", [128, S], BF16, isOutput=True)
        dbg_vs_d = nc.declare_dram_parameter(
            "dbg_vs", [128, NKC * 65], BF16, isOutput=True
        )
        dbg_ao_d = nc.declare_dram_parameter("dbg_ao", [128, 2, S], BF16, isOutput=True)

    with tile.TileContext(nc) as tc:
        with tc.tile_pool(name="glob", bufs=1) as glob:
            ktdup = glob.tile([128, S], BF16, tag="ktdup")
            v_s = glob.tile([128, NKC, 65], BF16, tag="v_s")
            outA = glob.tile([128, S], BF16, tag="outA")
            outB = glob.tile([128, S], BF16, tag="outB")
            ao = glob.tile([128, 2, S], BF16, tag="ao")
            ident = glob.tile([128, 128], BF16, tag="ident")
            wo_s = glob.tile([128, 2, D], BF16, tag="wo_s")
            cos_s = glob.tile([128, S], BF16, tag="cos_s")
            sin_s = glob.tile([128, S], BF16, tag="sin_s")
            wq_s = glob.tile([128, DCH, 384], BF16, tag="wq_s")
            kvraw = glob.tile([128, S], BF16, tag="kvraw")
            kswap = glob.tile([64, S], BF16, tag="kswap")
            qs01 = glob.tile([128, S], BF16, tag="qs01")
            qs23 = glob.tile([128, S], BF16, tag="qs23")
            rcp_t = glob.tile([128, 2, QBW], F32, tag="rcp_t")
            rcp_b = glob.tile([128, 2, QBW], BF16, tag="rcp_b")
            ones_b = glob.tile([128, 128], BF16, tag="ones_b")

            nc.vector.memset(v_s[:, :, 64], 1.0)
            nc.vector.memset(ones_b[:], 1.0)

            with (
                tc.tile_pool(name="xp", bufs=2) as xp,
                tc.tile_pool(name="aab", bufs=4) as aab,
                tc.tile_pool(name="tmpp", bufs=4) as tmpp,
                tc.tile_pool(name="pttp", bufs=6) as pttp,
                tc.tile_pool(name="stgp", bufs=6) as stgp,
                tc.tile_pool(name="yp", bufs=4) as yp,
                tc.tile_pool(name="psc", bufs=2, space="PSUM") as pscp,
                tc.tile_pool(name="pso", bufs=2, space="PSUM") as psop,
                tc.tile_pool(name="pse", bufs=2, space="PSUM") as psep,
            ):
                xt_r = xt_d.rearrange("(ko p) s -> p ko s", p=128)
                wqkv_r = wqkv_d.rearrange("(ko p) n -> p ko n", p=128)
                xblks = {}

                def kick_x(sb):
                    sbc = slice(sb * SBW, (sb + 1) * SBW)
                    xblk = xp.tile([128, DCH, SBW], BF16, tag="xblk")
                    for g in range(4):
                        nc.sync.dma_start(
                            xblk[:, 4 * g : 4 * g + 4, :], xt_r[:, 4 * g : 4 * g + 4, sbc]
                        )
                    xblks[sb] = xblk

                def kick_tabs(sb):
                    sbc = slice(sb * SBW, (sb + 1) * SBW)
                    nc.sync.dma_start(cos_s[:, sbc], cos_d[:, sbc])
                    nc.sync.dma_start(sin_s[:, sbc], sin_d[:, sbc])

                # ---------------- attention unit -------------------------
                def emit_unit(
                    pair, qb, qs, hooks=(), head=None, head2=None, early_hooks=()
                ):
                    q0 = qb * QBW
                    nkc = 4 * (qb + 1)
                    hooks = list(hooks)
                    pso_E = psop.tile([128, QBW], F32, tag="pso")
                    pso_O = psop.tile([128, QBW], F32, tag="pso")
                    ptts = {}

                    def scores(c):
                        kc0 = c * 128
                        d = max(0, kc0 - q0)
                        psc = pscp.tile([128, 2, 512], F32, tag="psc")
                        ptt = pttp.tile([128, 2, 512], BF16, tag="ptt")
                        nc.tensor.matmul(
                            psc[:, 0, d:512],
                            lhsT=ktdup[0:64, kc0 : kc0 + 128],
                            rhs=qs[0:64, q0 + d : q0 + QBW],
                            start=True,
                            stop=True,
                            tile_position=(0, 0),
                        )
                        nc.tensor.matmul(
                            psc[:, 1, d:512],
                            lhsT=ktdup[64:128, kc0 : kc0 + 128],
                            rhs=qs[64:128, q0 + d : q0 + QBW],
                            start=True,
                            stop=True,
                            tile_position=(64, 0),
                        )
                        nc.scalar.activation(
                            ptt[:, :, d:512], psc[:, :, d:512], AF.Exp
                        )
                        if kc0 + 127 > q0:
                            ww = min(512, (kc0 - q0) + 128)
                            for half in (0, 1):
                                sl = slice(d, ww)
                                nc.gpsimd.affine_select(
                                    out=ptt[:, half, sl],
                                    in_=ptt[:, half, sl],
                                    compare_op=ALU.is_ge,
                                    fill=0.0,
                                    base=q0 - kc0 + d,
                                    channel_multiplier=-1,
                                    pattern=[[1, ww - d]],
                                )
                        ptts[c] = ptt

                    def av(c):
                        kc0 = c * 128
                        d = max(0, kc0 - q0)
                        ptt = ptts.pop(c)
                        nc.tensor.matmul(
                            pso_E[0:65, d:QBW],
                            lhsT=v_s[:, c, :],
                            rhs=ptt[:, 0, d:512],
                            start=(c == 0),
                            stop=(c == nkc - 1),
                        )
                        nc.tensor.matmul(
                            pso_O[0:65, d:QBW],
                            lhsT=v_s[:, c, :],
                            rhs=ptt[:, 1, d:512],
                            start=(c == 0),
                            stop=(c == nkc - 1),
                        )

                    scores(0)
                    if nkc > 1:
                        scores(1)
                    av(0)
                    if head is not None:
                        head()
                    hi = 0
                    ei = 0
                    early_hooks = list(early_hooks)
                    # ao-dependent hooks (o_proj) may read ao written by head2
                    # (fired at c==2): first position must be >= 3
                    pos = (
                        [
                            min(3 + (i * max(0, nkc - 3)) // len(hooks), nkc - 1)
                            for i in range(len(hooks))
                        ]
                        if hooks
                        else []
                    )
                    for c in range(1, nkc):
                        if c + 1 < nkc:
                            scores(c + 1)
                        av(c)
                        if ei < len(early_hooks):
                            early_hooks[ei]()
                            ei += 1
                        if c == 2 and head2 is not None:
                            head2()
                        while hi < len(hooks) and pos[hi] <= c:
                            hooks[hi]()
                            hi += 1
                    while ei < len(early_hooks):
                        early_hooks[ei]()
                        ei += 1
                    while hi < len(hooks):
                        hooks[hi]()
                        hi += 1

                    def fin1():
                        # DVE-only: reciprocal of exp-sums + bf16 cast
                        nc.vector.reciprocal_approx_fast(rcp_t[:, 0, :], pso_E[:, :])
                        nc.vector.tensor_copy(rcp_b[64:65, 0, :], rcp_t[64:65, 0, :])
                        nc.vector.reciprocal_approx_fast(rcp_t[:, 1, :], pso_O[:, :])
                        nc.vector.tensor_copy(rcp_b[64:65, 1, :], rcp_t[64:65, 1, :])

                    def fin2():
                        # broadcast MM + normalize + eviction; emitted a couple
                        # of chunks after fin1 so the PE never waits on DVE
                        ch = pair
                        qsl = slice(q0, q0 + QBW)
                        bcps = pscp.tile([128, 2, 512], F32, tag="psc")
                        nc.tensor.matmul(
                            bcps[0:64, 0, :],
                            lhsT=ones_b[64:65, 0:64],
                            rhs=rcp_b[64:65, 0, :],
                            start=True,
                            stop=True,
                        )
                        nc.tensor.matmul(
                            bcps[0:64, 1, :],
                            lhsT=ones_b[64:65, 0:64],
                            rhs=rcp_b[64:65, 1, :],
                            start=True,
                            stop=True,
                        )
                        bcE_s = stgp.tile([64, QBW], F32, tag="bcs")
                        nc.vector.tensor_copy(bcE_s[0:64, :], bcps[0:64, 0, :])
                        nc.vector.tensor_tensor(
                            ao[0:64, ch, qsl], pso_E[0:64, :], bcE_s[0:64, :], ALU.mult
                        )
                        bcO_s = stgp.tile([64, QBW], F32, tag="bcs")
                        nc.vector.tensor_copy(bcO_s[0:64, :], bcps[0:64, 1, :])
                        stg = stgp.tile([64, QBW], BF16, tag="stg")
                        nc.vector.tensor_tensor(
                            stg[0:64, :], pso_O[0:64, :], bcO_s[0:64, :], ALU.mult
                        )
                        nc.gpsimd.dma_start(ao[64:128, ch, qsl], stg[0:64, :])

                    return fin1, fin2

                # ---------------- o_proj -------------------------------
                def oproj_half(st, hb):
                    """o_proj for s-chunk st (128 rows of y), half hb (1024 cols)."""
                    ysb = yp.tile([128, 1024], BF16, tag="ysb")
                    on_dve = (st * 2 + hb) % 2 == 0
                    for obi in (0, 1):
                        ob = 2 * hb + obi
                        pt = psep.tile([128, QBW], F32, tag="pse")
                        for chn in (0, 1):
                            nc.tensor.matmul(
                                pt[:],
                                lhsT=ao[:, chn, st * 128 : (st + 1) * 128],
                                rhs=wo_s[:, chn, ob * 512 : (ob + 1) * 512],
                                start=(chn == 0),
                                stop=(chn == 1),
                            )
                        dst = ysb[:, obi * 512 : (obi + 1) * 512]
                        if on_dve:
                            nc.vector.tensor_copy(dst, pt[:])
                        else:
                            nc.scalar.activation(dst, pt[:], AF.Copy)
                    eng = nc.gpsimd if on_dve else nc.scalar
                    eng.dma_start(
                        y_d[st * 128 : (st + 1) * 128, hb * 1024 : (hb + 1) * 1024],
                        ysb[:],
                    )

                # ---------------- projection step ----------------------
                def proj_mm(sb):
                    sbc = slice(sb * SBW, (sb + 1) * SBW)
                    xblk = xblks[sb]
                    psA = psep.tile([128, SBW], F32, tag="pse")
                    psB = psep.tile([128, SBW], F32, tag="pse")
                    aA = aab.tile([128, SBW], BF16, tag="aab")
                    aB = aab.tile([128, SBW], BF16, tag="aab")
                    for ps_t, a_t, col0 in ((psA, aA, 0), (psB, aB, 128)):
                        for kc in range(DCH):
                            nc.tensor.matmul(
                                ps_t[:],
                                lhsT=wq_s[:, kc, col0 : col0 + 128],
                                rhs=xblk[:, kc, :],
                                start=(kc == 0),
                                stop=(kc == DCH - 1),
                            )
                        nc.scalar.activation(a_t[:], ps_t[:], AF.Copy)
                    psKV = psep.tile([128, SBW], F32, tag="pse")
                    for kc in range(DCH):
                        nc.tensor.matmul(
                            psKV[:],
                            lhsT=wq_s[:, kc, 256:384],
                            rhs=xblk[:, kc, :],
                            start=(kc == 0),
                            stop=(kc == DCH - 1),
                        )
                    nc.scalar.activation(kvraw[:, sbc], psKV[:], AF.Copy)
                    return aA, aB

                def proj_post(sb, aA, aB):
                    sbc = slice(sb * SBW, (sb + 1) * SBW)
                    # RoPE on q (A = first-half dims, B = second halves)
                    tmp1 = tmpp.tile([128, SBW], BF16, tag="tmp")
                    tmp2 = tmpp.tile([128, SBW], BF16, tag="tmp")
                    nc.vector.tensor_tensor(
                        outA[:, sbc], aA[:], cos_s[:, sbc], ALU.mult
                    )
                    nc.gpsimd.tensor_tensor(tmp1[:], aB[:], sin_s[:, sbc], ALU.mult)
                    nc.vector.tensor_tensor(
                        outA[:, sbc], outA[:, sbc], tmp1[:], ALU.subtract
                    )
                    nc.vector.tensor_tensor(
                        outB[:, sbc], aB[:], cos_s[:, sbc], ALU.mult
                    )
                    nc.gpsimd.tensor_tensor(tmp2[:], aA[:], sin_s[:, sbc], ALU.mult)
                    nc.vector.tensor_tensor(
                        outB[:, sbc], outB[:, sbc], tmp2[:], ALU.add
                    )
                    # k RoPE via swapped halves
                    nc.sync.dma_start(kswap[0:32, sbc], kvraw[32:64, sbc])
                    nc.sync.dma_start(kswap[32:64, sbc], kvraw[0:32, sbc])
                    tmpk = tmpp.tile([64, SBW], BF16, tag="tmpk")
                    nc.vector.tensor_tensor(
                        ktdup[0:64, sbc], kvraw[0:64, sbc], cos_s[0:64, sbc], ALU.mult
                    )
                    nc.gpsimd.tensor_tensor(
                        tmpk[:], kswap[:, sbc], sin_s[0:64, sbc], ALU.mult
                    )
                    nc.vector.tensor_tensor(
                        ktdup[0:32, sbc], ktdup[0:32, sbc], tmpk[0:32, :], ALU.subtract
                    )
                    nc.vector.tensor_tensor(
                        ktdup[32:64, sbc], ktdup[32:64, sbc], tmpk[32:64, :], ALU.add
                    )
                    nc.sync.dma_start(ktdup[64:128, sbc], ktdup[0:64, sbc])
                    # v: [64, 512] -> 4 key-chunk tiles [128, 64] via PE transpose
                    for c in range(4 * sb, 4 * sb + 4):
                        ptr = pscp.tile([128, 2, 512], F32, tag="psc")
                        ptrb = ptr.bitcast(BF16)
                        nc.tensor.transpose(
                            ptrb[:, 0, 0:64],
                            kvraw[64:128, c * 128 : (c + 1) * 128],
                            ident[64:128, 64:128],
                        )
                        nc.vector.tensor_copy(v_s[:, c, 0:64], ptrb[:, 0, 0:64])
                    # qs streams for this s-block (both pairs)
                    nc.sync.dma_start(qs01[0:32, sbc], outA[0:32, sbc])
                    nc.sync.dma_start(qs01[32:64, sbc], outB[0:32, sbc])
                    nc.sync.dma_start(qs01[64:96, sbc], outA[32:64, sbc])
                    nc.sync.dma_start(qs01[96:128, sbc], outB[32:64, sbc])
                    nc.sync.dma_start(qs23[0:32, sbc], outA[64:96, sbc])
                    nc.sync.dma_start(qs23[32:64, sbc], outB[64:96, sbc])
                    nc.sync.dma_start(qs23[64:96, sbc], outA[96:128, sbc])
                    nc.sync.dma_start(qs23[96:128, sbc], outB[96:128, sbc])

                def make_proj_hooks(sb):
                    cell = {}

                    def mm():
                        cell["ab"] = proj_mm(sb)

                    def post():
                        proj_post(sb, *cell["ab"])

                    return mm, post

                def oproj_hooks(qb):
                    return [
                        lambda st=st, hb=hb: oproj_half(st, hb)
                        for st in range(4 * qb, 4 * qb + 4)
                        for hb in (0, 1)
                    ]

                # ---------------- driver --------------------------------
                # prologue: fine-grained alternating wq/x pieces on the two
                # DMA rings so data completes in kc order (the first proj
                # matmuls consume kc sequentially; coarse batches would all
                # share HBM bandwidth and finish together)
                sbc0 = slice(0, SBW)
                xblk0 = xp.tile([128, DCH, SBW], BF16, tag="xblk")
                xblks[0] = xblk0
                for p in range(8):
                    nc.gpsimd.dma_start(
                        wq_s[:, 2 * p : 2 * p + 2, :], wqkv_r[:, 2 * p : 2 * p + 2, :]
                    )
                    nc.sync.dma_start(
                        xblk0[:, 2 * p : 2 * p + 2, :], xt_r[:, 2 * p : 2 * p + 2, sbc0]
                    )
                make_identity(nc, ident[:])
                aAB0 = proj_mm(0)
                # non-critical loads: kicked after proj(0) so their transfers
                # don't steal HBM bandwidth from the first projection
                kick_tabs(0)
                for chn in range(2):
                    nc.gpsimd.dma_start(wo_s[:, chn, :], wo_d[chn])
                kick_x(1)
                kick_tabs(1)
                proj_post(0, *aAB0)

                fin1 = fin2 = None
                for s in (1, 2, 3):
                    if s < 3:
                        kick_x(s + 1)
                        kick_tabs(s + 1)
                    qb = s - 1
                    oh = oproj_hooks(qb - 1) if qb >= 1 else []
                    pm, pp = make_proj_hooks(s)
                    # pm inside unitA, pp early in unitB: the RoPE/qs chain of
                    # block s then overlaps the rest of the step instead of
                    # gating the next step's units
                    fin1, fin2 = emit_unit(
                        0,
                        qb,
                        qs01,
                        hooks=oh[:4],
                        head=fin1,
                        head2=fin2,
                        early_hooks=[pm],
                    )
                    fin1, fin2 = emit_unit(
                        1,
                        qb,
                        qs23,
                        hooks=oh[4:],
                        head=fin1,
                        head2=fin2,
                        early_hooks=[pp],
                    )

                # P2: qb=3 units + remaining o_proj
                oh = oproj_hooks(2)
                fin1, fin2 = emit_unit(0, 3, qs01, hooks=oh[:4], head=fin1, head2=fin2)
                fin1, fin2 = emit_unit(1, 3, qs23, hooks=oh[4:], head=fin1, head2=fin2)
                fin1()
                fin2()
                for st in range(12, 16):
                    for hb in (0, 1):
                        oproj_half(st, hb)
                if kdbg:
                    nc.sync.dma_start(dbg_qs_d[:, 0, :], qs01[:])
                    nc.sync.dma_start(dbg_qs_d[:, 1, :], qs23[:])
                    nc.sync.dma_start(dbg_kt_d[:], ktdup[:])
                    nc.sync.dma_start(
                        dbg_vs_d.rearrange("p (c n) -> p c n", c=NKC), v_s[:]
                    )
                    nc.sync.dma_start(dbg_ao_d[:], ao[:])
    nc.compile()
    return nc


def _prep_inputs(x, Wq, Wk, Wv, Wo, inv_freq):
    """Host-side sharding + layout prep. Returns in_maps for the 8 cores."""
    x = np.ascontiguousarray(np.asarray(x, dtype=np.float32).reshape(S, D))
    xt = np.ascontiguousarray(x.T)  # [D, S]

    pos = np.arange(S, dtype=np.float64)
    inv = np.asarray(inv_freq, dtype=np.float64)  # [32]
    freqs = pos[None, :] * inv[:, None]  # [32, S]
    cos32 = np.cos(freqs).astype(np.float32)
    sin32 = np.sin(freqs).astype(np.float32)
    cos_tab = np.tile(cos32, (4, 1))  # [128, S]
    sin_tab = np.tile(sin32, (4, 1))

    in_maps = []
    for i in range(NCORES):
        wq_l = Wq[256 * i : 256 * (i + 1)].astype(np.float32) * 0.125  # [256, D]
        wk_l = Wk[64 * i : 64 * (i + 1)].astype(np.float32)  # [64, D]
        wv_l = Wv[64 * i : 64 * (i + 1)].astype(np.float32)  # [64, D]
        # A-tile: first-half dims of the 4 heads; B-tile: second halves
        wA = np.concatenate(
            [wq_l[64 * h : 64 * h + 32] for h in range(HQ)], axis=0
        )  # [128, D]
        wB = np.concatenate(
            [wq_l[64 * h + 32 : 64 * h + 64] for h in range(HQ)], axis=0
        )
        wkv = np.concatenate([wk_l, wv_l], axis=0)  # [128, D]
        wqkv = np.ascontiguousarray(
            np.concatenate([wA, wB, wkv], axis=0).T
        )  # [D, 384]
        wo_l = Wo[:, 256 * i : 256 * (i + 1)].astype(np.float32)  # [D, 256]
        wo_t = np.ascontiguousarray(wo_l.T.reshape(2, 128, D))  # [2, 128, D]
        in_maps.append(
            {
                "xt": xt.astype(ml_dtypes.bfloat16),
                "wqkv": wqkv.astype(ml_dtypes.bfloat16),
                "wo": wo_t.astype(ml_dtypes.bfloat16),
                "cos": cos_tab.astype(ml_dtypes.bfloat16),
                "sin": sin_tab.astype(ml_dtypes.bfloat16),
            }
        )
    return in_maps


_NC_CACHE = None


def kernel(x, Wq, Wk, Wv, Wo, inv_freq):
    global _NC_CACHE
    if _NC_CACHE is None:
        _NC_CACHE = _build_nc()
    nc = _NC_CACHE
    in_maps = _prep_inputs(x, Wq, Wk, Wv, Wo, inv_freq)
    trace = bool(int(os.environ.get("BASS_KERNEL_TRACE", "0")))
    res = None
    last_exc = None
    for attempt in range(3):
        try:
            res = run_bass_kernel_spmd(nc, in_maps, list(range(NCORES)), trace=trace)
            break
        except Exception as e:  # transient device faults (rare) — retry
            last_exc = e
            msg = str(e)
            if "UNRECOVERABLE" in msg or "UNAVAILABLE" in msg or "Timeout" in msg:
                continue
            raise
    if res is None:
        raise last_exc
    if trace:
        kernel.last_results = res
    y = np.zeros((S, D), dtype=np.float32)
    for i in range(NCORES):
        y += res.results[i]["y"].astype(np.float32)
    return y.reshape(1, S, D)
